# revision 4
# baseline (speedup 1.0000x reference)
"""Trainium2 Bass kernel for ContinuousDGM message passing.

  xe = x @ W_emb + b_emb            [N, E]
  D  = sq_cdist(xe)                 [N, N]
  A  = 1 / (1 + D)
  W  = A / A.sum(axis=1)            (broadcast over last axis -> col-normalize)
  out = W @ xe                      [N, E]

Strategy (8 NeuronCores, row-block sharding, fully fused -- the [N,N]
matrices never touch DRAM):
  * Host passes x already transposed (xT [DIN, N]) plus the core's own
    column block (xTl), so every matmul has its contraction dim on
    partitions with no on-device transpose of x.
  * On device, augmented operand buffers augL/augR [68, N] bf16 hold
    [-2*xeT; ones; ones; sq_hi; sq_lo] so ONE matmul produces
    psum = 1 + sq_i + sq_j - 2*G = 1 + D for any tile.  sq rides as a
    bf16 hi+lo pair (error ~3e-4) and is computed from the *same* bf16
    xe values the PE multiplies, so the diagonal cancels to ~1e-3
    without any masking.
  * A = reciprocal(psum) on DVE (approx_fast, ~18 bits).
  * Pass 1: local row sums s (symmetry => col sums), tiny AllGather of
    1/s (4KB/core).  Pass 2: recompute A^T tiles, matmul with
    ye = xe * (1/s) accumulating out^T [E, B] per core in PSUM.
  * Host concatenates the 8 out^T blocks and transposes.
"""

import os
import sys

import numpy as np

N, DIN, E = 8192, 256, 64
P = 128
C = 8
B = N // C            # 1024 rows per core
SUP = 512
NSUP = N // SUP       # 16
BSUP = B // SUP       # 2
NT = N // P           # 64
BT = B // P           # 8

_NC_CACHE = {}


def _import_concourse():
    try:
        import concourse.bacc  # noqa: F401
    except ImportError:
        for p in ("/opt/trn_rl_repo", "/root/.axon_site/_ro/trn_rl_repo"):
            if os.path.isdir(p) and p not in sys.path:
                sys.path.insert(0, p)
        import concourse.bacc  # noqa: F401


def build_body(tc, outT, xT, xTl, W, b, eye):
    """Emit the kernel body. All args are bass APs of DRAM tensors."""
    from contextlib import ExitStack

    import concourse.bass as bass  # noqa: F401
    from concourse import mybir

    nc = tc.nc
    f32 = mybir.dt.float32
    bf16 = mybir.dt.bfloat16
    AF = mybir.ActivationFunctionType
    ALU = mybir.AluOpType
    AX = mybir.AxisListType

    with ExitStack() as ctx:
        big = ctx.enter_context(tc.tile_pool(name="big", bufs=1))
        const = ctx.enter_context(tc.tile_pool(name="const", bufs=1))
        work = ctx.enter_context(tc.tile_pool(name="work", bufs=1))
        psum = ctx.enter_context(tc.tile_pool(name="psum", bufs=1, space="PSUM"))
        dram = ctx.enter_context(tc.tile_pool(name="dram", bufs=1, space="DRAM"))

        # ---------- load inputs ----------
        xk = [big.tile([P, N], f32, name=f"xk{t}", tag=f"xk{t}") for t in range(2)]
        CH = 2048
        for t in range(2):
            for ci in range(N // CH):
                nc.sync.dma_start(
                    xk[t][:, ci * CH:(ci + 1) * CH],
                    xT[t * P:(t + 1) * P, ci * CH:(ci + 1) * CH],
                )
        xlk = [big.tile([P, B], f32, name=f"xlk{t}", tag=f"xlk{t}") for t in range(2)]
        for t in range(2):
            nc.sync.dma_start(xlk[t][:], xTl[t * P:(t + 1) * P, :])

        Wsb = const.tile([P, 2, E], f32, name="Wsb", tag="Wsb")
        for t in range(2):
            nc.sync.dma_start(Wsb[:, t, :], W[t * P:(t + 1) * P, :])
        b_col = const.tile([E, 1], f32, name="bcol", tag="bcol")
        nc.sync.dma_start(b_col[:], b[:])
        b2_col = const.tile([E, 1], f32, name="b2col", tag="b2col")
        nc.vector.tensor_scalar_mul(b2_col[:], b_col[:], -2.0)
        eye_f = const.tile([P, P], f32, name="eyef", tag="eyef")
        nc.sync.dma_start(eye_f[:], eye[:])
        eye_b = const.tile([P, P], bf16, name="eyeb", tag="eyeb")
        nc.scalar.copy(eye_b[:], eye_f[:])

        # ---------- augmented operand buffers ----------
        # augL rows: [0:64]=-2*xeT, [64:66]=1, [66:68]=sq hi/lo   (stationary side)
        # augR rows: [0:64]=xeT, [64:66]=(sq+1) hi/lo, [66:68]=1  (moving side)
        augL = big.tile([68, N], bf16, name="augL", tag="augL")
        augR = big.tile([68, N], bf16, name="augR", tag="augR")
        augLl = big.tile([68, B], bf16, name="augLl", tag="augLl")
        augRl = big.tile([68, B], bf16, name="augRl", tag="augRl")
        # engine ops need partition starts in {0,32,64,96}; DMA is exempt,
        # so stage the ones rows in a [2, N] tile and DMA them into place.
        onesrow = work.tile([2, N], bf16, name="onesrow", tag="onesrow")
        nc.vector.memset(onesrow[:], 1.0)
        nc.sync.dma_start(augL[64:66, :], onesrow[:])
        nc.sync.dma_start(augR[66:68, :], onesrow[:])
        nc.sync.dma_start(augLl[64:66, :], onesrow[:, 0:B])
        nc.sync.dma_start(augRl[66:68, :], onesrow[:, 0:B])

        # xeT supers: psum[e, i] = sum_k W[k, e] * x[i, k]  (+ b via ACT bias)
        def emit_xeT(dst_R, dst_L, xsrc, nsup):
            for s in range(nsup):
                ps = psum.tile([E, SUP], f32, name="p64", tag="p64", bufs=2)
                for t in range(2):
                    nc.tensor.matmul(
                        ps[:],
                        lhsT=Wsb[:, t, :],
                        rhs=xsrc[t][:, s * SUP:(s + 1) * SUP],
                        start=(t == 0),
                        stop=(t == 1),
                    )
                sl = slice(s * SUP, (s + 1) * SUP)
                nc.scalar.activation(dst_R[0:64, sl], ps[:], AF.Identity,
                                     bias=b_col[:], scale=1.0)
                nc.scalar.activation(dst_L[0:64, sl], ps[:], AF.Identity,
                                     bias=b2_col[:], scale=-2.0)

        emit_xeT(augR, augL, xk, NSUP)
        emit_xeT(augRl, augLl, xlk, BSUP)

        # ---------- row-major bf16 xe + sq (from the SAME bf16 values) ----------
        xe_bf = big.tile([P, NT * E], bf16, name="xebf", tag="xebf")
        sq_mat = const.tile([P, NT], f32, name="sqmat", tag="sqmat")
        for it in range(NT):
            pt = psum.tile([P, E], bf16, name="pT", tag="pT", bufs=2)
            nc.tensor.transpose(pt[:], augR[0:64, it * P:(it + 1) * P],
                                eye_b[0:64, 0:64])
            nc.vector.tensor_copy(out=xe_bf[:, it * E:(it + 1) * E], in_=pt[:])
            junkE = work.tile([P, E], bf16, name="junkE", tag="junkE", bufs=2)
            nc.scalar.activation(junkE[:], pt[:], AF.Square,
                                 accum_out=sq_mat[:, it:it + 1])
        sql_mat = const.tile([P, BT], f32, name="sqlmat", tag="sqlmat")
        for it in range(BT):
            pt = psum.tile([P, E], bf16, name="pT", tag="pT", bufs=2)
            nc.tensor.transpose(pt[:], augRl[0:64, it * P:(it + 1) * P],
                                eye_b[0:64, 0:64])
            junkE = work.tile([P, E], bf16, name="junkE", tag="junkE", bufs=2)
            nc.scalar.activation(junkE[:], pt[:], AF.Square,
                                 accum_out=sql_mat[:, it:it + 1])

        # ---------- sq rows (hi/lo bf16) -> aug rows ----------
        def sq_rows(sq_tile, nt, dst_L, dst_R, nelem):
            # sq_tile [128, nt] -> T [nt, 128] -> hi/lo splits -> DMA into rows
            pt = psum.tile([nt, P], f32, name="pT2", tag="pT2", bufs=1)
            nc.tensor.transpose(pt[:], sq_tile[:], eye_f[:])
            T = work.tile([nt, P], f32, name="Tf", tag="Tf", bufs=2)
            nc.scalar.copy(T[:], pt[:])

            def hilo(src, dst0, dst1):
                hi = work.tile([nt, P], bf16, name="hi", tag="hi", bufs=2)
                nc.scalar.copy(hi[:], src[:])
                hif = work.tile([nt, P], f32, name="hif", tag="hif", bufs=2)
                nc.vector.tensor_copy(out=hif[:], in_=hi[:])
                lo = work.tile([nt, P], f32, name="lo", tag="lo", bufs=2)
                nc.vector.tensor_tensor(lo[:], src[:], hif[:], ALU.subtract)
                lob = work.tile([nt, P], bf16, name="lob", tag="lob", bufs=2)
                nc.scalar.copy(lob[:], lo[:])
                nc.sync.dma_start(dst0, hi[:])
                nc.sync.dma_start(dst1, lob[:])

            # stationary side: sq
            hilo(T, dst_L[66:67, 0:nelem], dst_L[67:68, 0:nelem])
            # moving side: sq + 1
            Tn = work.tile([nt, P], f32, name="Tn", tag="Tn", bufs=2)
            nc.vector.tensor_scalar_add(Tn[:], T[:], 1.0)
            hilo(Tn, dst_R[64:65, 0:nelem], dst_R[65:66, 0:nelem])

        sq_rows(sq_mat, NT, augL, augR, N)
        sq_rows(sql_mat, BT, augLl, augRl, B)

        # ---------- pass 1: s = row sums of A over local rows ----------
        s_loc = const.tile([P, BT], f32, name="sloc", tag="sloc")
        for it in range(BT):
            sparts = work.tile([P, NSUP], f32, name="sparts", tag="sparts", bufs=2)
            for js in range(NSUP):
                pg = psum.tile([P, SUP], f32, name="pA", tag="pA", bufs=3)
                nc.tensor.matmul(pg[:],
                                 lhsT=augLl[:, it * P:(it + 1) * P],
                                 rhs=augR[:, js * SUP:(js + 1) * SUP],
                                 start=True, stop=True)
                ar = work.tile([P, SUP], f32, name="ar", tag="ar", bufs=3)
                nc.vector.reciprocal_approx_fast(out=ar[:], in_=pg[:])
                junk1 = work.tile([P, SUP], bf16, name="junk1", tag="junk1", bufs=2)
                nc.scalar.activation(junk1[:], ar[:], AF.Copy,
                                     accum_out=sparts[:, js:js + 1])
            nc.vector.tensor_reduce(s_loc[:, it:it + 1], sparts[:],
                                    axis=AX.X, op=ALU.add)

        rs_loc = const.tile([P, BT], f32, name="rsloc", tag="rsloc")
        nc.vector.reciprocal(rs_loc[:], s_loc[:])
        prt = psum.tile([BT, P], f32, name="pT2", tag="pT2", bufs=1)
        nc.tensor.transpose(prt[:], rs_loc[:], eye_f[:])
        rs_row = work.tile([BT, P], f32, name="rsrow", tag="rsrow", bufs=1)
        nc.scalar.copy(rs_row[:], prt[:])

        # ---------- AllGather 1/s ----------
        ag_in = dram.tile([B], f32, name="agin", tag="agin")
        ag_out = dram.tile([N], f32, name="agout", tag="agout", addr_space="Shared")
        nc.sync.dma_start(ag_in[:], rs_row[:])
        nc.gpsimd.collective_compute(
            "AllGather", ALU.bypass,
            replica_groups=[list(range(C))],
            ins=[ag_in[:]], outs=[ag_out[:]],
        )
        rs_full = work.tile([NT, P], f32, name="rsfull", tag="rsfull", bufs=1)
        nc.sync.dma_start(rs_full[:], ag_out[:])
        prc = psum.tile([P, NT], f32, name="pT", tag="pT", bufs=2)
        nc.tensor.transpose(prc[:], rs_full[:], eye_f[0:64, 0:64])
        rs_col = const.tile([P, NT], f32, name="rscol", tag="rscol")
        nc.scalar.copy(rs_col[:], prc[:])

        # ---------- ye = xe * (1/s) ----------
        ye_bf = big.tile([P, NT * E], bf16, name="yebf", tag="yebf")
        for jt in range(NT):
            sl = slice(jt * E, (jt + 1) * E)
            nc.vector.tensor_scalar_mul(ye_bf[:, sl], xe_bf[:, sl],
                                        rs_col[:, jt:jt + 1])

        # ---------- pass 2: out^T[e, i] = sum_j ye[j, e] * A[j, i] ----------
        for isup in range(BSUP):
            po = psum.tile([E, SUP], f32, name="p64", tag="p64", bufs=2)
            for jt in range(NT):
                pg = psum.tile([P, SUP], f32, name="pA", tag="pA", bufs=3)
                nc.tensor.matmul(pg[:],
                                 lhsT=augL[:, jt * P:(jt + 1) * P],
                                 rhs=augRl[:, isup * SUP:(isup + 1) * SUP],
                                 start=True, stop=True)
                at = work.tile([P, SUP], f32, name="ar", tag="ar", bufs=3)
                nc.vector.reciprocal_approx_fast(out=at[:], in_=pg[:])
                atb = work.tile([P, SUP], bf16, name="atb", tag="atb", bufs=3)
                nc.scalar.copy(atb[:], at[:])
                nc.tensor.matmul(po[:],
                                 lhsT=ye_bf[:, jt * E:(jt + 1) * E],
                                 rhs=atb[:],
                                 start=(jt == 0), stop=(jt == NT - 1))
            osb = work.tile([E, SUP], f32, name="osb", tag="osb", bufs=2)
            nc.scalar.copy(osb[:], po[:])
            nc.sync.dma_start(outT[:, isup * SUP:(isup + 1) * SUP], osb[:])


def _build_nc():
    _import_concourse()
    import concourse.bacc as bacc
    import concourse.tile as tile
    from concourse import mybir

    f32 = mybir.dt.float32
    nc = bacc.Bacc("TRN2", target_bir_lowering=False, debug=False,
                   num_devices=C)
    xT = nc.dram_tensor("xT", [DIN, N], f32, kind="ExternalInput").ap()
    xTl = nc.dram_tensor("xTl", [DIN, B], f32, kind="ExternalInput").ap()
    W = nc.dram_tensor("W", [DIN, E], f32, kind="ExternalInput").ap()
    b = nc.dram_tensor("b", [E, 1], f32, kind="ExternalInput").ap()
    eye = nc.dram_tensor("eye", [P, P], f32, kind="ExternalInput").ap()
    outT = nc.dram_tensor("outT", [E, B], f32, kind="ExternalOutput").ap()

    with tile.TileContext(nc) as tc:
        build_body(tc, outT, xT, xTl, W, b, eye)
    nc.compile()
    return nc


def make_in_maps(x, W_emb, b_emb):
    xT = np.ascontiguousarray(x.T).astype(np.float32)
    eye = np.eye(P, dtype=np.float32)
    bb = np.asarray(b_emb, dtype=np.float32).reshape(E, 1)
    Wf = np.asarray(W_emb, dtype=np.float32)
    in_maps = []
    for c in range(C):
        in_maps.append({
            "xT": xT,
            "xTl": np.ascontiguousarray(xT[:, c * B:(c + 1) * B]),
            "W": Wf,
            "b": bb,
            "eye": eye,
        })
    return in_maps


def kernel(x, W_emb, b_emb, _trace=False, _tmpdir=None):
    _import_concourse()
    from concourse import bass_utils

    key = "nc"
    if key not in _NC_CACHE:
        _NC_CACHE[key] = _build_nc()
    nc = _NC_CACHE[key]

    in_maps = make_in_maps(np.asarray(x), np.asarray(W_emb), np.asarray(b_emb))
    res = bass_utils.run_bass_kernel_spmd(
        nc, in_maps, core_ids=list(range(C)),
        trace=_trace, tmpdir=_tmpdir,
    )
    blocks = [np.asarray(res.results[c]["outT"]) for c in range(C)]
    outT = np.concatenate(blocks, axis=1)          # [E, N]
    out = np.ascontiguousarray(outT.T).astype(np.float32)  # [N, E]
    if _trace:
        return out, res
    return out
